# revision 19
# baseline (speedup 1.0000x reference)
"""Trainium2 Bass kernel for nn_MultiHeadCrossAttention (B=4, S=1024, D=1024,
H=16, Hd=64), 8 NeuronCores.

Sharding: 8 cores = 4 batches x 2 half-head groups. Core 2b+s handles batch b,
global heads s*8..s*8+7, and produces side s's output (s=0: query_out, s=1:
key_out). The two attention directions share exp(E) per head: each core
computes its heads' energies ONCE (halving the scalar-engine exp work vs a
per-side sharding), uses the [k-part, q] layout for its own side's attention
(path-1) and a DMA-xbar transposed copy [q-part, k] for the partner side's
attention (path-2). Path-2 results are exchanged pairwise via chunked
AllReduce(+) with local subtraction (keeps the SPMD program parity-free).
The fc contraction-slot order is per-core host-packed into wf so the program
never indexes by parity.

Precision: fp8(e4m3) inputs/weights with x32 weight scaling (avoids fp8
subnormals), DoubleRow fp8 matmuls (2 k-tiles per pass) for the projections
and fc, bf16 exp/attention path. Residual+LN runs at 1024x scale (LN is scale
invariant; eps is pre-scaled).
"""
import sys
import types

import ml_dtypes
import numpy as np

BF16 = ml_dtypes.bfloat16
F8 = ml_dtypes.float8_e4m3

try:  # noqa: SIM105
    if "antenv.axon_hooks" not in sys.modules:
        from trn_agent_boot.trn_boot import _ntff_profile_via_ctypes

        _m = types.ModuleType("antenv.axon_hooks")
        _hook = _ntff_profile_via_ctypes("/opt/axon/libaxon_pjrt.so")
        _m.get_axon_ntff_profile_hook = lambda: _hook
        sys.modules["antenv.axon_hooks"] = _m
except Exception:
    pass

import concourse.bacc as bacc
import concourse.mybir as mybir
import concourse.tile as tile
from concourse.bass_utils import run_bass_kernel_spmd

P = 128
D = 1024
S = 1024
HL = 8          # local heads per core
NHP = 4         # local head pairs
HD = 64
NC = D // P     # 8 contraction chunks
NI = NC // 2    # 4 DoubleRow chunk pairs
WS = 32.0       # weight scale (fp8 subnormal dodge)
EXPSCALE = 0.125 / (WS * WS)
EPS_SCALED = 1e-5 * (WS * WS) * (WS * WS)  # LN at 1024x scale

f32 = mybir.dt.float32
bf16 = mybir.dt.bfloat16
fp8 = mybir.dt.float8e4
ADD = mybir.AluOpType.add
SUB = mybir.AluOpType.subtract
MUL = mybir.AluOpType.mult
EXP = mybir.ActivationFunctionType.Exp
SQRT = mybir.ActivationFunctionType.Sqrt
DR = mybir.MatmulPerfMode.DoubleRow

_CACHED_NC = None


def _body(tc, io):
    nc = tc.nc
    (at_d, bt_d, vt_d, wa_d, wb_d, wv_d, wf_d, ba2_d, bb2_d, bvb_d, ares_d,
     out_d) = io

    with tc.tile_pool(name="consts", bufs=1) as consts, \
         tc.tile_pool(name="vpool", bufs=1) as vpool, \
         tc.tile_pool(name="wfp", bufs=1) as wfp, \
         tc.tile_pool(name="abp", bufs=2) as abp, \
         tc.tile_pool(name="ptp", bufs=2) as ptp, \
         tc.tile_pool(name="xtp", bufs=1) as xtp, \
         tc.tile_pool(name="xsp", bufs=2) as xsp, \
         tc.tile_pool(name="stp", bufs=2) as stp, \
         tc.tile_pool(name="denp", bufs=1) as denp, \
         tc.tile_pool(name="rbp", bufs=1) as rbp, \
         tc.tile_pool(name="dram", bufs=4, space="DRAM") as dram:
        ba2_sb = consts.tile([P, NHP], f32)
        bb2_sb = consts.tile([P, NHP], f32)
        bvb_sb = consts.tile([P, 512], f32, tag="bvb")
        ones_sb = consts.tile([P, 1], bf16, tag="ones")
        ones_row = consts.tile([1, HD], bf16, tag="onesr")
        nc.any.memset(ones_sb[:], 1.0)
        nc.any.memset(ones_row[:], 1.0)

        v_sb = vpool.tile([P, NC, HL, HD], bf16)
        wf_sb = wfp.tile([P, NI, 2, D], fp8, tag="wf")
        xt_sb = xtp.tile([P, NI, 2, S], fp8, tag="xt")

        dens_sb = denp.tile([1, S], f32, tag="dens")
        denf_sb = denp.tile([1, S], f32, tag="denf")
        denr_sb = denp.tile([1, 2 * S], bf16, tag="denr")
        denr2_sb = denp.tile([1, 2 * S], bf16, tag="denr2")
        rb1_sb = rbp.tile([P, S], f32, tag="rb1")
        rb2_sb = rbp.tile([P, S], f32, tag="rb2")

        a_c = {}
        b_c = {}
        pexp = {}
        pexpT = {}
        xts = {}
        bnc_in = {}
        bnc_out = {}

        with tc.tile_pool(name="inp", bufs=1) as inp, \
             tc.tile_pool(name="pexpp", bufs=11) as pexpp, \
             tc.tile_pool(name="pxps", bufs=2, space="PSUM") as px_ps, \
             tc.tile_pool(name="epsp", bufs=3, space="PSUM") as eps_ps:
            at_sb = inp.tile([P, NC, S], fp8, tag="at")
            bt_sb = inp.tile([P, NC, S], fp8, tag="bt")
            vt_sb = inp.tile([P, NC, S], fp8, tag="vt")
            wa_sb = inp.tile([P, NI, 2, 512], fp8, tag="wa")
            wb_sb = inp.tile([P, NI, 2, 512], fp8, tag="wb")
            wv_sb = inp.tile([P, NI, 2, 512], fp8, tag="wv")

            # startup DMAs split per-chunk so transfers round-robin across HW
            # DMA rings; sync is free until the first transposes (iter 1)
            for i in range(NI):
                nc.scalar.dma_start(wa_sb[:, i], wa_d[:, i])
                nc.scalar.dma_start(wb_sb[:, i], wb_d[:, i])
            nc.scalar.dma_start(ba2_sb[:], ba2_d)
            nc.scalar.dma_start(bb2_sb[:], bb2_d)
            for dc in range(NC):
                nc.sync.dma_start(at_sb[:, dc, :], at_d[dc * P:(dc + 1) * P, :])
                nc.sync.dma_start(bt_sb[:, dc, :], bt_d[dc * P:(dc + 1) * P, :])
            for dc in range(NC):
                nc.scalar.dma_start(vt_sb[:, dc, :], vt_d[dc * P:(dc + 1) * P, :])
            for i in range(NI):
                nc.scalar.dma_start(wv_sb[:, i], wv_d[:, i])
            nc.gpsimd.dma_start(bvb_sb[:], bvb_d)
            nc.gpsimd.dma_start(wf_sb[:], wf_d)

            def filler_ps():
                t = eps_ps.tile([P, 2 * 512], f32, tag="eps")
                return t[:, 0:512]

            def alloc_ab(c):
                a_c[c] = abp.tile([P, S], fp8, tag="a", name=f"a{c}")
                b_c[c] = abp.tile([P, S], fp8, tag="b", name=f"b{c}")

            def proj_group(c, which, sh):
                # q/k projection for head-pair c, seq-half sh: 4 DoubleRow MMs
                ps = filler_ps()
                w_t = wa_sb if which == 0 else wb_sb
                src = at_sb if which == 0 else bt_sb
                bias = ba2_sb if which == 0 else bb2_sb
                dst = a_c[c] if which == 0 else b_c[c]
                for i in range(NI):
                    nc.tensor.matmul(
                        ps,
                        w_t[:, i, :, c * P:(c + 1) * P],
                        src[:, 2 * i:2 * i + 2, sh * 512:(sh + 1) * 512],
                        start=(i == 0),
                        stop=(i == NI - 1),
                        perf_mode=DR,
                    )
                nc.vector.tensor_tensor(
                    out=dst[:, sh * 512:(sh + 1) * 512],
                    in0=ps,
                    in1=bias[:, c:c + 1].to_broadcast((P, 512)),
                    op=ADD,
                )

            def vproj_group(sc):
                ps = filler_ps()
                for i in range(NI):
                    nc.tensor.matmul(
                        ps,
                        vt_sb[:, 2 * i:2 * i + 2, sc * P:(sc + 1) * P],
                        wv_sb[:, i, :, :],
                        start=(i == 0),
                        stop=(i == NI - 1),
                        perf_mode=DR,
                    )
                nc.vector.tensor_tensor(
                    out=v_sb[:, sc, :, :],
                    in0=ps.rearrange("p (h d) -> p h d", d=HD),
                    in1=bvb_sb[:].rearrange("p (h d) -> p h d", d=HD),
                    op=ADD,
                )

            pd_t = {}

            def emit_den(c, path, half):
                # softmax denominators via ones-matmul partition reduction:
                # 4 concurrent M=1 col tiles (positions 0/32/64/96)
                if half == 0:
                    pd_t[(c, path)] = px_ps.tile([P, 512], f32, tag="px",
                                                 name=f"pd{c}_{path}")
                pd = pd_t[(c, path)]
                for u in range(half * 4, half * 4 + 4):
                    for h2 in range(2):
                        for w in range(2):
                            t = 2 * h2 + w
                            if path == 0:
                                rhs = pexp[(c, u)][:, h2, w, :]
                            else:
                                rhs = pexpT[c][:, u, h2, w * 512:(w + 1) * 512]
                            nc.tensor.matmul(
                                pd[32 * t:32 * t + 1, :],
                                ones_sb[:],
                                rhs,
                                start=(u == 0),
                                stop=(u == NC - 1),
                                skip_group_check=True,
                                tile_position=(0, 32 * t),
                            )
                if half == 1:
                    pdf = pd_t.pop((c, path))
                    denr = denr_sb if path == 0 else denr2_sb
                    rb = rb1_sb if path == 0 else rb2_sb
                    # reciprocal straight off the psum rows, then partition
                    # broadcast via K=1 ones-matmul (keeps gpsimd free: its
                    # in-order queue must not serialize behind collectives)
                    for h2 in range(2):
                        for w in range(2):
                            t = 2 * h2 + w
                            nc.vector.tensor_copy(
                                dens_sb[:, w * 512:(w + 1) * 512],
                                pdf[32 * t:32 * t + 1, :])
                        nc.vector.reciprocal_approx_fast(
                            out=denf_sb[:], in_=dens_sb[:])
                        nc.vector.tensor_copy(
                            denr[:, h2 * S:(h2 + 1) * S], denf_sb[:])
                    for w in range(2):
                        rb_ps = px_ps.tile([P, 512], f32, tag="px",
                                           name=f"rb{c}_{path}_{w}")
                        for h2 in range(2):
                            nc.tensor.matmul(
                                rb_ps[h2 * HD:(h2 + 1) * HD, :],
                                ones_row[0:1, :],
                                denr[:, h2 * S + w * 512:
                                     h2 * S + (w + 1) * 512],
                                start=True, stop=True,
                                skip_group_check=True,
                                tile_position=(0, h2 * HD),
                            )
                        nc.vector.tensor_copy(rb[:, w * 512:(w + 1) * 512],
                                              rb_ps[:])

            px_t = {}

            def emit_pv(c, path, half):
                # attention @ V, 2 concurrent M=64 col tiles per free-half
                for w in range(2):
                    if half == 0:
                        px_t[(path, w)] = px_ps.tile([P, 512], f32, tag="px",
                                                     name=f"px{c}_{path}_{w}")
                    px = px_t[(path, w)]
                    for u in range(half * 4, half * 4 + 4):
                        for h2 in range(2):
                            if path == 0:
                                rhs = pexp[(c, u)][:, h2, w, :]
                            else:
                                rhs = pexpT[c][:, u, h2, w * 512:(w + 1) * 512]
                            nc.tensor.matmul(
                                px[h2 * HD:(h2 + 1) * HD, :],
                                v_sb[:, u, 2 * c + h2, :],
                                rhs,
                                start=(u == 0),
                                stop=(u == NC - 1),
                                skip_group_check=True,
                            )

            def emit_muls(c, path):
                rb = rb1_sb if path == 0 else rb2_sb
                for w in range(2):
                    px = px_t.pop((path, w))
                    if path == 0:
                        dst = xt_sb[:, c // 2, c % 2, w * 512:(w + 1) * 512]
                    else:
                        dst = xts[c][:, w * 512:(w + 1) * 512]
                    nc.vector.tensor_tensor(
                        out=dst, in0=px[:],
                        in1=rb[:, w * 512:(w + 1) * 512], op=MUL)

            def emit_cc(c):
                # pairwise exchange of path-2 chunk: AllReduce(+) then local
                # subtract (parity-free program)
                bnc_in[c] = dram.tile([P, S], bf16, tag="bin", name=f"bin{c}")
                bnc_out[c] = dram.tile([P, S], bf16, tag="bout", name=f"bout{c}")
                nc.sync.dma_start(bnc_in[c][:], xts[c][:])
                nc.gpsimd.collective_compute(
                    "AllReduce", ADD,
                    replica_groups=[[0, 1], [2, 3], [4, 5], [6, 7]],
                    ins=[bnc_in[c].opt()],
                    outs=[bnc_out[c].opt()],
                )
                st = stp.tile([P, S], bf16, tag="stage", name=f"stage{c}")
                nc.sync.dma_start(st[:], bnc_out[c][:])
                nc.vector.tensor_tensor(
                    out=xt_sb[:, 2 + c // 2, c % 2, :],
                    in0=st[:], in1=xts[c][:], op=SUB)

            def consumers(cc, j):
                if j == 0:
                    emit_den(cc, 0, 0)
                elif j == 1:
                    emit_den(cc, 0, 1)
                elif j == 2:
                    emit_den(cc, 1, 0)
                elif j == 3:
                    emit_den(cc, 1, 1)
                elif j == 5:
                    emit_pv(cc, 0, 0)
                elif j == 7:
                    emit_pv(cc, 0, 1)
                elif j == 8:
                    emit_muls(cc, 0)
                elif j == 9:
                    xts[cc] = xsp.tile([P, S], bf16, tag="xts",
                                       name=f"xts{cc}")
                    emit_pv(cc, 1, 0)
                elif j == 11:
                    emit_pv(cc, 1, 1)
                elif j == 12:
                    emit_muls(cc, 1)
                elif j == 13:
                    emit_cc(cc)

            def emit_iteration(c, fillers):
                fi = 0

                def emit_fillers(n):
                    nonlocal fi
                    for _ in range(n):
                        if fi < len(fillers):
                            fillers[fi]()
                            fi += 1

                for j in range(16):
                    jc, ih = divmod(j, 2)
                    eps_t = eps_ps.tile([P, 2 * 512], f32, tag="eps",
                                        name=f"eps{c}_{jc}_{ih}")
                    for h2 in range(2):
                        off = h2 * HD
                        nc.tensor.matmul(
                            eps_t[:, h2 * 512:(h2 + 1) * 512],
                            b_c[c][off:off + HD, jc * P:(jc + 1) * P],
                            a_c[c][off:off + HD, ih * 512:(ih + 1) * 512],
                            start=True,
                            stop=True,
                        )
                    if ih == 0:
                        pexp[(c, jc)] = pexpp.tile([P, 2, 2, 512], bf16,
                                                   tag="pexp",
                                                   name=f"pexp{c}_{jc}")
                    nc.scalar.activation(pexp[(c, jc)][:, :, ih, :], eps_t[:],
                                         EXP, scale=EXPSCALE)
                    if ih == 1:
                        for h2 in range(2):
                            nc.sync.dma_start_transpose(
                                pexpT[c][:, :, h2, jc * P:(jc + 1) * P],
                                pexp[(c, jc)][:, h2])
                    if c >= 1:
                        consumers(c - 1, j)
                    if j in (4, 6, 10, 14, 15) or c == 0:
                        emit_fillers(1)
                emit_fillers(len(fillers))

            alloc_ab(0)
            pexpT[0] = ptp.tile([P, NC, 2, S], bf16, tag="pexpT", name="pT0")
            for sh in range(2):
                for which in range(2):
                    proj_group(0, which, sh)
            for c in range(NHP):
                fillers = []
                if c + 1 < NHP:
                    alloc_ab(c + 1)
                    pexpT[c + 1] = ptp.tile([P, NC, 2, S], bf16, tag="pexpT",
                                            name=f"pT{c + 1}")
                    for sh in range(2):
                        for which in range(2):
                            fillers.append(
                                lambda c_=c + 1, w_=which, sh_=sh:
                                proj_group(c_, w_, sh_))
                if c == 0:
                    for sc in range(NC):
                        fillers.append(lambda sc_=sc: vproj_group(sc_))
                emit_iteration(c, fillers)
            # drain: consumers of the last head pair, k-path first so its
            # collective fires as early as possible
            emit_den(3, 1, 0)
            emit_den(3, 1, 1)
            xts[3] = xsp.tile([P, S], bf16, tag="xts", name="xts3")
            emit_pv(3, 1, 0)
            emit_pv(3, 1, 1)
            emit_muls(3, 1)
            emit_cc(3)
            emit_den(3, 0, 0)
            emit_den(3, 0, 1)
            emit_pv(3, 0, 0)
            emit_pv(3, 0, 1)
            emit_muls(3, 0)

        # ---- tail: fc + residual + LN, per q-chunk (input/pexp pools closed
        # above so these reuse their SBUF) ---------------------------------
        with tc.tile_pool(name="pxps2", bufs=2, space="PSUM") as px2_ps, \
             tc.tile_pool(name="aresp", bufs=3) as aresp, \
             tc.tile_pool(name="ph3", bufs=2) as ph3:
                for ic in range(NC):
                    ares_t = aresp.tile([P, D], bf16, tag="ares")
                    nc.scalar.dma_start(ares_t[:],
                                        ares_d[ic * P:(ic + 1) * P, :])
                    z_t = ph3.tile([P, D], f32, tag="z")
                    dump_t = ph3.tile([P, 512], f32, tag="dump")
                    qsum = [ph3.tile([P, 1], f32, tag=f"qs{dh}",
                                     name=f"qs{ic}_{dh}")
                            for dh in range(2)]
                    zps = [px2_ps.tile([P, 512], f32, tag="px2",
                                       name=f"z{ic}_{dh}") for dh in range(2)]
                    for i in range(NI):
                        for dh in range(2):
                            nc.tensor.matmul(
                                zps[dh],
                                xt_sb[:, i, :, ic * P:(ic + 1) * P],
                                wf_sb[:, i, :, dh * 512:(dh + 1) * 512],
                                start=(i == 0),
                                stop=(i == NI - 1),
                                perf_mode=DR,
                            )
                    for dh in range(2):
                        sl = slice(dh * 512, (dh + 1) * 512)
                        nc.vector.tensor_tensor(
                            out=z_t[:, sl], in0=zps[dh], in1=ares_t[:, sl],
                            op=ADD)
                        nc.scalar.activation(
                            dump_t[:], z_t[:, sl],
                            mybir.ActivationFunctionType.Square,
                            accum_out=qsum[dh][:])
                    mean_t = ph3.tile([P, 1], f32, tag="mean")
                    var_t = ph3.tile([P, 1], f32, tag="var")
                    msq_t = ph3.tile([P, 1], f32, tag="msq")
                    sd_t = ph3.tile([P, 1], f32, tag="sd")
                    rstd_t = ph3.tile([P, 1], f32, tag="rstd")
                    mrs_t = ph3.tile([P, 1], f32, tag="mrs")
                    nc.vector.tensor_reduce(
                        out=mean_t[:], in_=z_t[:], axis=mybir.AxisListType.X,
                        op=ADD)
                    nc.vector.tensor_scalar(
                        out=mean_t[:], in0=mean_t[:], scalar1=1.0 / D,
                        scalar2=None, op0=MUL)
                    nc.vector.tensor_tensor(
                        out=var_t[:], in0=qsum[0][:], in1=qsum[1][:], op=ADD)
                    nc.vector.tensor_scalar(
                        out=var_t[:], in0=var_t[:], scalar1=1.0 / D,
                        scalar2=EPS_SCALED, op0=MUL, op1=ADD)
                    nc.vector.tensor_tensor(
                        out=msq_t[:], in0=mean_t[:], in1=mean_t[:], op=MUL)
                    nc.vector.tensor_tensor(
                        out=var_t[:], in0=var_t[:], in1=msq_t[:], op=SUB)
                    nc.scalar.activation(sd_t[:], var_t[:], SQRT)
                    nc.vector.reciprocal(rstd_t[:], sd_t[:])
                    nc.vector.tensor_tensor(
                        out=mrs_t[:], in0=mean_t[:], in1=rstd_t[:], op=MUL)
                    o_t = ph3.tile([P, D], f32, tag="o")
                    nc.vector.tensor_scalar(
                        out=o_t[:], in0=z_t[:], scalar1=rstd_t[:],
                        scalar2=mrs_t[:], op0=MUL, op1=SUB)
                    nc.sync.dma_start(out_d[ic * P:(ic + 1) * P, :], o_t[:])


def _build():
    nc = bacc.Bacc(trn_type="TRN2", target_bir_lowering=False, debug=False,
                   num_devices=8)
    at_d = nc.dram_tensor("at", [D, S], fp8, kind="ExternalInput").ap()
    bt_d = nc.dram_tensor("bt", [D, S], fp8, kind="ExternalInput").ap()
    vt_d = nc.dram_tensor("vt", [D, S], fp8, kind="ExternalInput").ap()
    wa_d = nc.dram_tensor("wa", [P, NI, 2, 512], fp8, kind="ExternalInput").ap()
    wb_d = nc.dram_tensor("wb", [P, NI, 2, 512], fp8, kind="ExternalInput").ap()
    wv_d = nc.dram_tensor("wv", [P, NI, 2, 512], fp8, kind="ExternalInput").ap()
    wf_d = nc.dram_tensor("wf", [P, NI, 2, D], fp8, kind="ExternalInput").ap()
    ba2_d = nc.dram_tensor("ba2", [P, NHP], f32, kind="ExternalInput").ap()
    bb2_d = nc.dram_tensor("bb2", [P, NHP], f32, kind="ExternalInput").ap()
    bvb_d = nc.dram_tensor("bvb", [P, 512], f32, kind="ExternalInput").ap()
    ares_d = nc.dram_tensor("ares", [S, D], bf16, kind="ExternalInput").ap()
    out_d = nc.dram_tensor("out", [S, D], f32, kind="ExternalOutput").ap()
    io = (at_d, bt_d, vt_d, wa_d, wb_d, wv_d, wf_d, ba2_d, bb2_d, bvb_d,
          ares_d, out_d)
    with tile.TileContext(nc) as tc:
        _body(tc, io)
    nc.compile()
    return nc


def _get_nc():
    global _CACHED_NC
    if _CACHED_NC is None:
        _CACHED_NC = _build()
    return _CACHED_NC


def _pack_w(w_rows):
    # [512 out, 1024 din] -> [128 p, 4 i, 2 t, 512 o]; w[p,i,t,o] =
    # w_rows[o, (2i+t)*128+p]
    arr = np.asarray(w_rows, np.float32).T
    return np.ascontiguousarray(
        arr.reshape(NI, 2, P, 512).transpose(2, 0, 1, 3).astype(F8))


def kernel(query, key, value, Wq, bq, Wk, bk, Wv, bv, Wfq, bfq, Wfk, bfk,
           gamma_q, beta_q, gamma_k, beta_k):
    query = np.asarray(query, np.float32)
    key = np.asarray(key, np.float32)
    value = np.asarray(value, np.float32)
    B = query.shape[0]
    nc = _get_nc()

    Wq, Wk, Wv = np.asarray(Wq, np.float32), np.asarray(Wk, np.float32), \
        np.asarray(Wv, np.float32)
    Wfq, Wfk = np.asarray(Wfq, np.float32), np.asarray(Wfk, np.float32)
    bq_, bk_ = np.asarray(bq, np.float32), np.asarray(bk, np.float32)
    bv_ = np.asarray(bv, np.float32)
    bfq_, bfk_ = np.asarray(bfq, np.float32), np.asarray(bfk, np.float32)

    side_consts = []
    for s in range(2):
        Wa, ba = (Wq, bq_) if s == 0 else (Wk, bk_)
        Wb, bb = (Wk, bk_) if s == 0 else (Wq, bq_)
        WF = Wfq if s == 0 else Wfk
        rows = slice(s * 512, (s + 1) * 512)
        bases = [s * 512 + j * P for j in range(4)] + \
            [(1 - s) * 512 + j * P for j in range(4)]
        cols = np.concatenate([np.arange(b0, b0 + P) for b0 in bases])
        arrf = (WS * WF)[:, cols].T  # [1024 slot-din, 1024 out]
        side_consts.append(dict(
            wa=_pack_w(WS * Wa[rows]),
            wb=_pack_w(WS * Wb[rows]),
            wv=_pack_w(WS * Wv[rows]),
            wf=np.ascontiguousarray(
                arrf.reshape(NI, 2, P, D).transpose(2, 0, 1, 3).astype(F8)),
            ba2=np.ascontiguousarray((WS * ba[rows]).reshape(NHP, P).T),
            bb2=np.ascontiguousarray((WS * bb[rows]).reshape(NHP, P).T),
            bvb=np.ascontiguousarray(np.broadcast_to(
                WS * bv_[rows], (P, 512)).astype(np.float32)),
        ))

    in_maps = []
    for b in range(B):
        for s in range(2):
            A = query[b] if s == 0 else key[b]
            Bx = key[b] if s == 0 else query[b]
            bf_side = bfq_ if s == 0 else bfk_
            sc = side_consts[s]
            in_maps.append({
                "at": np.ascontiguousarray(A.T.astype(F8)),
                "bt": np.ascontiguousarray(Bx.T.astype(F8)),
                "vt": np.ascontiguousarray(value[b].T.astype(F8)),
                "wa": sc["wa"], "wb": sc["wb"], "wv": sc["wv"],
                "wf": sc["wf"],
                "ba2": sc["ba2"], "bb2": sc["bb2"], "bvb": sc["bvb"],
                "ares": np.ascontiguousarray(
                    ((WS * WS) * (A + bf_side)).astype(BF16)),
            })

    res = run_bass_kernel_spmd(nc, in_maps, core_ids=list(range(len(in_maps))))
    global _LAST_EXEC_NS, _LAST_RES
    _LAST_EXEC_NS = res.exec_time_ns
    _LAST_RES = res
    query_out = np.stack([res.results[2 * b]["out"] for b in range(B)])
    key_out = np.stack([res.results[2 * b + 1]["out"] for b in range(B)])

    gq = np.asarray(gamma_q, np.float32); bq2 = np.asarray(beta_q, np.float32)
    gk = np.asarray(gamma_k, np.float32); bk2 = np.asarray(beta_k, np.float32)
    if not (np.all(gq == 1.0) and np.all(bq2 == 0.0)):
        query_out = query_out * gq + bq2
    if not (np.all(gk == 1.0) and np.all(bk2 == 0.0)):
        key_out = key_out * gk + bk2
    return (query_out, key_out)
